# revision 5
# baseline (speedup 1.0000x reference)
"""BiModal attention kernel for Trainium2 (8 NeuronCores, data-parallel over batch).

Per core (one batch b): x, y: [2048, 128] fp32.
  S = x @ y.T                    (float32r matmuls, [2048, 2048])
  E = exp(S)                     (unshifted; softmax is shift-invariant and
                                  |S| <~ 67 so exp stays in fp32/bf16 range)
  a1 = (E @ y) / rowsum(E) * x
  a2 = (E.T @ x) / colsum(E) * y
  out = concat([a1, a2], -1)     ([2048, 256])

Layout: rows are relabeled r = 16*p + b (p = SBUF partition, b = block index)
so every DRAM transfer is contiguous per partition; the relabeling is applied
consistently to s and t everywhere, so the math is unchanged.

v2 schedule: one fused pipeline instead of phase-serial execution.
 - x^T / y^T built with PE transposes (f32) during the load stream; this also
   warms the PE so HAM grants full clock by the time the main loop is dense.
 - Main loop (2 column panels x 16 row blocks): S matmuls (f32r, full-rate at
   512-wide) -> exp on ACT (1024-wide, fused row-sum accum) -> DMA-xbar
   transpose of E into ET.  o2 accumulates per-block right behind exp (lag 1);
   o1 runs as per-s-chunk bursts (16-deep contraction into one PSUM bank) as
   soon as the needed ET columns land; epilogues (PE re-transpose + DVE
   gating + batched stores) stream inside panel 1.
 - l2 column sums: per-(tb, half-row) DVE/GPSIMD reduces with scalar output
   (enables the 2x/4x DVE modes the strided-output variant forfeits).
PSUM budget (8 banks): S rotation 2x[128,1024] (4) + o2 accum [128,2,512] (2)
+ o1-burst/epilogue rotation 2x[128,512] (2).
"""
import sys

sys.path.insert(0, "/opt/trn_rl_repo")

import os
import numpy as np

import concourse.bass as bass
import concourse.mybir as mybir
import concourse.tile as tile
from concourse.tile_rust import add_dep_helper
from concourse import bacc
from concourse.bass_utils import run_bass_kernel_spmd
from concourse.masks import make_identity

f32 = mybir.dt.float32
f32r = mybir.dt.float32r
bf16 = mybir.dt.bfloat16

B = 8
S = 2048
D = 128
P = 128
NB = S // P          # 16 row/col blocks
NP = 2               # column panels
PW = S // NP         # panel width (1024)
PB = PW // P         # blocks per panel (8)

_NC_CACHE = None
LAST_EXEC_NS = None


def _build_program(nc):
    x_d = nc.dram_tensor("x", [S, D], f32, kind="ExternalInput").ap()
    y_d = nc.dram_tensor("y", [S, D], f32, kind="ExternalInput").ap()
    out_d = nc.dram_tensor("out", [S, 2 * D], f32, kind="ExternalOutput").ap()

    # contiguous-per-partition views; row r = 16*p + b
    x_dv = x_d.rearrange("(p b) d -> p b d", p=P)      # [128, 16, 128]
    y_dv = y_d.rearrange("(p b) d -> p b d", p=P)
    out_dv = out_d.rearrange("(p b) c -> p b c", p=P)  # [128, 16, 256]

    Exp = mybir.ActivationFunctionType.Exp
    MUL = mybir.AluOpType.mult
    ADD = mybir.AluOpType.add
    AX = mybir.AxisListType.X

    # per-engine no-sync order chains: engine streams execute exactly in
    # emission order, so the interleaving below is the schedule.
    last = {}

    def seq(key, inst):
        prev = last.get(key)
        if prev is not None:
            add_dep_helper(inst.ins, prev.ins, sync=False, reason="order")
        last[key] = inst
        return inst

    with tile.TileContext(nc) as tc:
        with (
            tc.tile_pool(name="sb", bufs=1) as sb,
            tc.tile_pool(name="stg", bufs=4) as stg,
            tc.tile_pool(name="ps", bufs=1, space="PSUM") as ps,
        ):
            # ---- persistent SBUF tensors ----
            x_sb = sb.tile([P, NB, D], f32, tag="x_sb")
            y_sb = sb.tile([P, NB, D], f32, tag="y_sb")
            xT = sb.tile([P, NB, P], f32r, tag="xT")       # [d, sb, sp]
            yT = sb.tile([P, NB, P], f32r, tag="yT")       # [d, tb, tp]
            x_hi = sb.tile([P, NB, D], bf16, tag="x_hi")
            y_hi = sb.tile([P, NB, D], bf16, tag="y_hi")
            E = sb.tile([P, NB, S], bf16, tag="E")         # [sp, sb, t-pos]
            ET = sb.tile([P, NB, S], bf16, tag="ET")       # [tp, tb, s-pos]
            o1T_sb = sb.tile([P, S], f32, tag="o1T")       # [d, s-pos]
            o2T_sb = sb.tile([P, S], f32, tag="o2T")       # [d, t-pos]
            ident = sb.tile([P, P], f32, tag="ident")
            l1p = sb.tile([P, 2 * NB], f32, tag="l1p")     # [sp, 2*i+ct]
            l1c = sb.tile([P, NB], f32, tag="l1c")
            r1 = sb.tile([P, NB], f32, tag="r1")
            l2h = sb.tile([P, NB, 2], f32, tag="l2h")      # [tp, tb, s-half]
            l2 = sb.tile([P, NB], f32, tag="l2")
            r2 = sb.tile([P, NB], f32, tag="r2")

            make_identity(nc, ident[:])

            # ---- PSUM tiles (8 banks total) ----
            sA = [ps.tile([P, PW], f32, tag="A0", name="sA0"),
                  ps.tile([P, PW], f32, tag="A1", name="sA1")]
            o2_ps = ps.tile([P, 2, 512], f32, tag="B", name="o2_ps")
            c_ps = [ps.tile([P, 512], f32, tag="C0", name="c_ps0"),
                    ps.tile([P, 512], f32, tag="C1", name="c_ps1")]

            # ---- loads: 4-block chunks, panel-0 critical data first ----
            def load(dv, sbuf, c):
                seq("sp", nc.sync.dma_start(sbuf[:, 4 * c:4 * c + 4],
                                            dv[:, 4 * c:4 * c + 4]))

            load(x_dv, x_sb, 0)
            load(y_dv, y_sb, 0)
            load(y_dv, y_sb, 1)
            load(x_dv, x_sb, 1)
            load(x_dv, x_sb, 2)
            load(x_dv, x_sb, 3)
            load(y_dv, y_sb, 2)
            load(y_dv, y_sb, 3)

            # ---- prologue: xT/yT via PE transposes (f32, exact) ----
            # each [128,128] block transposes into a PSUM slice; drains cast
            # f32 -> f32r on ACT (idle before exps) and DVE.
            def ptr(v_sb, b, pslice):
                seq("pe", nc.tensor.transpose(pslice, v_sb[:, b, :], ident[:]))

            for b in range(4):                   # x0-3 -> A0[0:512]
                ptr(x_sb, b, sA[0][:, b * P:(b + 1) * P])
            for b in range(4):                   # y0-3 -> B[0, 0:512]
                ptr(y_sb, b, o2_ps[:, 0, b * P:(b + 1) * P])
            for b in range(4, 8):                # y4-7 -> B[1, 0:512]
                ptr(y_sb, b, o2_ps[:, 1, (b - 4) * P:(b - 3) * P])
            for b in range(4, 8):                # x4-7 -> A0[512:1024]
                ptr(x_sb, b, sA[0][:, b * P:(b + 1) * P])
            for b in range(8, 16):               # x8-15 -> A1
                ptr(x_sb, b, sA[1][:, (b - 8) * P:(b - 7) * P])
            for b in range(8, 12):               # y8-11 -> C0
                ptr(y_sb, b, c_ps[0][:, (b - 8) * P:(b - 7) * P])
            for b in range(12, 16):              # y12-15 -> C1
                ptr(y_sb, b, c_ps[1][:, (b - 12) * P:(b - 11) * P])

            # drains (f32 psum -> f32r SBUF)
            seq("act", nc.scalar.copy(xT[:, 0:4, :], sA[0][:, 0:512]))
            seq("act", nc.scalar.copy(yT[:, 0:4, :], o2_ps[:, 0, :]))
            seq("act", nc.scalar.copy(yT[:, 4:8, :], o2_ps[:, 1, :]))
            seq("dve", nc.vector.tensor_scalar_add(xT[:, 4:8, :],
                                                   sA[0][:, 512:1024], 0.0))
            seq("dve", nc.vector.tensor_scalar_add(xT[:, 8:12, :],
                                                   sA[1][:, 0:512], 0.0))
            seq("dve", nc.vector.tensor_scalar_add(xT[:, 12:16, :],
                                                   sA[1][:, 512:1024], 0.0))
            seq("dve", nc.vector.tensor_scalar_add(yT[:, 8:12, :],
                                                   c_ps[0][:], 0.0))
            seq("dve", nc.vector.tensor_scalar_add(yT[:, 12:16, :],
                                                   c_ps[1][:], 0.0))

            # bf16 copies of x/y (matmul operands for o2/o1) on GPSIMD
            def conv(dst, src, c):
                seq("gps", nc.gpsimd.tensor_scalar_add(
                    dst[:, 4 * c:4 * c + 4, :], src[:, 4 * c:4 * c + 4, :], 0.0))

            conv(x_hi, x_sb, 0)
            conv(y_hi, y_sb, 0)
            conv(y_hi, y_sb, 1)
            conv(x_hi, x_sb, 1)
            conv(x_hi, x_sb, 2)
            conv(x_hi, x_sb, 3)
            conv(y_hi, y_sb, 2)
            conv(y_hi, y_sb, 3)

            yT_f = yT[:].rearrange("p b d -> p (b d)")

            # ---- emission helpers ----
            def s_block(ct, i):
                c0 = ct * PW
                slot = sA[i % 2]
                seq("pe", nc.tensor.matmul(slot[:, 0:512], xT[:, i, :],
                                           yT_f[:, c0:c0 + 512],
                                           start=True, stop=True))
                seq("pe", nc.tensor.matmul(slot[:, 512:1024], xT[:, i, :],
                                           yT_f[:, c0 + 512:c0 + 1024],
                                           start=True, stop=True))

            def exp_block(ct, i):
                c0 = ct * PW
                idx = 2 * i + ct
                seq("act", nc.scalar.activation(
                    E[:, i, c0:c0 + PW], sA[i % 2][:], Exp,
                    accum_out=l1p[:, idx:idx + 1]))

            def xbar(ct, i):
                c0 = ct * PW
                seq("sp", nc.sync.dma_start_transpose(
                    ET[:, ct * PB:(ct + 1) * PB, i * P:(i + 1) * P],
                    E[:, i, c0:c0 + PW]))

            def o2_block(ct, i):
                c0 = ct * PW
                for q in range(2):
                    seq("pe", nc.tensor.matmul(
                        o2_ps[:, q, :], x_hi[:, i, :],
                        E[:, i, c0 + q * 512:c0 + (q + 1) * 512],
                        start=(i == 0), stop=(i == NB - 1)))

            def o1_burst(ct, j, bank):
                # s-chunk j (cols j*512..), contract tb over panel ct's blocks
                for k in range(PB):
                    tb = ct * PB + k
                    seq("pe", nc.tensor.matmul(
                        bank[:], y_hi[:, tb, :],
                        ET[:, tb, j * 512:(j + 1) * 512],
                        start=(k == 0), stop=(k == PB - 1)))

            def o1_drain(ct, j, bank):
                dst = o1T_sb[:, j * 512:(j + 1) * 512]
                if ct == 0:
                    seq("dve", nc.vector.tensor_scalar_add(dst, bank[:], 0.0))
                else:
                    seq("dve", nc.vector.tensor_tensor(dst, bank[:], dst, op=ADD))

            def l1_chunk(j):
                # combine panel sums + reciprocal for s-blocks 4j..4j+3
                src = l1p[:, 8 * j:8 * j + 8].rearrange("p (i c) -> p i c", c=2)
                seq("dve", nc.vector.tensor_reduce(l1c[:, 4 * j:4 * j + 4], src,
                                                   axis=AX, op=ADD))
                seq("dve", nc.vector.reciprocal(r1[:, 4 * j:4 * j + 4],
                                                l1c[:, 4 * j:4 * j + 4]))

            def l2_half(tb, h):
                seq("dve", nc.vector.tensor_reduce(
                    l2h[:, tb, h:h + 1],
                    ET[:, tb, h * PW:(h + 1) * PW], axis=AX, op=ADD))

            def l2_chunk(k):
                seq("dve", nc.vector.tensor_reduce(l2[:, 4 * k:4 * k + 4],
                                                   l2h[:, 4 * k:4 * k + 4, :],
                                                   axis=AX, op=ADD))
                seq("dve", nc.vector.reciprocal(r2[:, 4 * k:4 * k + 4],
                                                l2[:, 4 * k:4 * k + 4]))

            def epi(which, k, bank):
                # 4 blocks: PE transpose + DVE gate into staging, 1 store
                src_sb = o1T_sb if which == 1 else o2T_sb
                rcp = r1 if which == 1 else r2
                gate = x_sb if which == 1 else y_sb
                col0 = 0 if which == 1 else D
                st = stg.tile([P, 4, D], f32, tag="st", name=f"st{which}_{k}")
                for q in range(4):
                    blk = 4 * k + q
                    seq("pe", nc.tensor.transpose(
                        bank[:, q * P:(q + 1) * P],
                        src_sb[:, blk * P:(blk + 1) * P], ident[:]))
                    seq("dve", nc.vector.scalar_tensor_tensor(
                        st[:, q, :], bank[:, q * P:(q + 1) * P],
                        rcp[:, blk:blk + 1], gate[:, blk, :],
                        op0=MUL, op1=MUL))
                seq("sp", nc.sync.dma_start(
                    out_dv[:, 4 * k:4 * k + 4, col0:col0 + D], st[:]))

            # ---- main loop: 2 panels x 16 blocks, fully fused ----
            for ct in range(NP):
                for i in range(NB):
                    s_block(ct, i)
                    if i >= 1:
                        o2_block(ct, i - 1)
                    if ct == 1:
                        if i == 0:
                            o2_block(0, NB - 1)       # finish panel-0 o2
                            # drain panel-0 o2 before o2(1,0) reuses the bank
                            seq("dve", nc.vector.tensor_scalar_add(
                                o2T_sb[:, 0:PW],
                                o2_ps[:].rearrange("p a b -> p (a b)"), 0.0))
                        if i == 1:
                            o1_burst(0, 3, c_ps[1])   # last h0 burst
                        if i == 2:
                            o1_drain(0, 3, c_ps[1])
                        if i == 3:
                            # l2 second halves for panel-0 tb's, then r2
                            for kk in range(PB):
                                l2_half(kk, 1)
                            l2_chunk(0)
                            l2_chunk(1)
                        if i == 5:
                            epi(2, 0, c_ps[1])
                        if i == 9:
                            epi(2, 1, c_ps[1])
                    if i in (6, 10, 14):
                        j = (i - 6) // 4
                        o1_burst(ct, j, c_ps[j % 2])
                    if i in (7, 11, 15):
                        j = (i - 7) // 4
                        o1_drain(ct, j, c_ps[j % 2])
                        if ct == 1:
                            l1_chunk(j)
                            epi(1, j, c_ps[j % 2])
                    if i >= 8:
                        # l2 first halves for this panel's tb's
                        l2_half(ct * PB + (i - 8), 0)
                    exp_block(ct, i)
                    xbar(ct, i)

            # ---- tail ----
            o2_block(1, NB - 1)
            seq("dve", nc.vector.tensor_scalar_add(
                o2T_sb[:, PW:S], o2_ps[:].rearrange("p a b -> p (a b)"), 0.0))
            o1_burst(1, 3, c_ps[1])
            o1_drain(1, 3, c_ps[1])
            l1_chunk(3)
            for kk in range(PB):
                l2_half(PB + kk, 1)
            l2_chunk(2)
            l2_chunk(3)
            epi(1, 3, c_ps[1])
            epi(2, 2, c_ps[0])
            epi(2, 3, c_ps[0])

    nc.compile()
    return nc


def _get_nc():
    global _NC_CACHE
    if _NC_CACHE is None:
        nc = bacc.Bacc("TRN2", target_bir_lowering=False, debug=False,
                       num_devices=B)
        _NC_CACHE = _build_program(nc)
    return _NC_CACHE


def kernel(x, y):
    global LAST_EXEC_NS
    nc = _get_nc()
    x = np.asarray(x, dtype=np.float32)
    y = np.asarray(y, dtype=np.float32)
    in_maps = [
        {"x": np.ascontiguousarray(x[b]), "y": np.ascontiguousarray(y[b])}
        for b in range(B)
    ]
    trace = bool(int(os.environ.get("KERNEL_TRACE", "0")))
    res = run_bass_kernel_spmd(nc, in_maps, list(range(B)), trace=trace)
    LAST_EXEC_NS = res.exec_time_ns
    return np.stack([res.results[b]["out"] for b in range(B)], axis=0)


# revision 8
# speedup vs baseline: 1.1063x; 1.1063x over previous
"""BiModal attention kernel for Trainium2 (8 NeuronCores, data-parallel over batch).

Per core (one batch b): x, y: [2048, 128] fp32.
  S = x @ y.T                    (float32r matmuls, [2048, 2048])
  E = exp(S)                     (unshifted; softmax is shift-invariant and
                                  |S| <~ 67 so exp stays in fp32/bf16 range)
  a1 = (E @ y) / rowsum(E) * x
  a2 = (E.T @ x) / colsum(E) * y
  out = concat([a1, a2], -1)     ([2048, 256])

Layout: rows are relabeled r = 16*p + b (p = SBUF partition, b = block index)
so every DRAM transfer is contiguous per partition; the relabeling is applied
consistently to s and t everywhere, so the math is unchanged.

v2 schedule: one fused pipeline instead of phase-serial execution.
 - x^T / y^T built with PE transposes (f32) during the load stream; this also
   warms the PE so HAM grants full clock by the time the main loop is dense.
 - Main loop (2 column panels x 16 row blocks): S matmuls (f32r, full-rate at
   512-wide) -> exp on ACT (1024-wide, fused row-sum accum) -> DMA-xbar
   transpose of E into ET.  o2 accumulates per-block right behind exp (lag 1);
   o1 runs as per-s-chunk bursts (16-deep contraction into one PSUM bank) as
   soon as the needed ET columns land; epilogues (PE re-transpose + DVE
   gating + batched stores) stream inside panel 1.
 - l2 column sums: per-(tb, half-row) DVE/GPSIMD reduces with scalar output
   (enables the 2x/4x DVE modes the strided-output variant forfeits).
PSUM budget (8 banks): S rotation 2x[128,1024] (4) + o2 accum [128,2,512] (2)
+ o1-burst/epilogue rotation 2x[128,512] (2).
"""
import sys

sys.path.insert(0, "/opt/trn_rl_repo")

import os
import numpy as np

import concourse.bass as bass
import concourse.mybir as mybir
import concourse.tile as tile
from concourse.tile_rust import add_dep_helper
from concourse import bacc
from concourse.bass_utils import run_bass_kernel_spmd
from concourse.masks import make_identity

f32 = mybir.dt.float32
f32r = mybir.dt.float32r
bf16 = mybir.dt.bfloat16

B = 8
S = 2048
D = 128
P = 128
NB = S // P          # 16 row/col blocks
NP = 2               # column panels
PW = S // NP         # panel width (1024)
PB = PW // P         # blocks per panel (8)

_NC_CACHE = None
LAST_EXEC_NS = None


def _build_program(nc):
    x_d = nc.dram_tensor("x", [S, D], f32, kind="ExternalInput").ap()
    y_d = nc.dram_tensor("y", [S, D], f32, kind="ExternalInput").ap()
    out_d = nc.dram_tensor("out", [S, 2 * D], f32, kind="ExternalOutput").ap()

    # contiguous-per-partition views; row r = 16*p + b
    x_dv = x_d.rearrange("(p b) d -> p b d", p=P)      # [128, 16, 128]
    y_dv = y_d.rearrange("(p b) d -> p b d", p=P)
    out_dv = out_d.rearrange("(p b) c -> p b c", p=P)  # [128, 16, 256]

    Exp = mybir.ActivationFunctionType.Exp
    MUL = mybir.AluOpType.mult
    ADD = mybir.AluOpType.add
    AX = mybir.AxisListType.X

    # per-engine no-sync order chains: engine streams execute exactly in
    # emission order, so the interleaving below is the schedule.
    last = {}

    def seq(key, inst):
        prev = last.get(key)
        if prev is not None:
            add_dep_helper(inst.ins, prev.ins, sync=False, reason="order")
        last[key] = inst
        return inst

    with tile.TileContext(nc) as tc:
        with (
            tc.tile_pool(name="sb", bufs=1) as sb,
            tc.tile_pool(name="stg", bufs=4) as stg,
            tc.tile_pool(name="ps", bufs=1, space="PSUM") as ps,
        ):
            # ---- persistent SBUF tensors ----
            x_sb = sb.tile([P, NB, D], f32, tag="x_sb")
            y_sb = sb.tile([P, NB, D], f32, tag="y_sb")
            xT = sb.tile([P, NB, P], f32r, tag="xT")       # [d, sb, sp]
            yT = sb.tile([P, NB, P], f32r, tag="yT")       # [d, tb, tp]
            x_hi = sb.tile([P, NB, D], bf16, tag="x_hi")
            y_hi = sb.tile([P, NB, D], bf16, tag="y_hi")
            E = sb.tile([P, NB, S], bf16, tag="E")         # [sp, sb, t-pos]
            ET = sb.tile([P, NB, S], bf16, tag="ET")       # [tp, tb, s-pos]
            o1T_sb = sb.tile([P, S], f32, tag="o1T")       # [d, s-pos]
            o2T_sb = sb.tile([P, S], f32, tag="o2T")       # [d, t-pos]
            ident = sb.tile([P, P], f32, tag="ident")
            l1p = sb.tile([P, 2 * NB], f32, tag="l1p")     # [sp, 2*i+ct]
            l1c = sb.tile([P, NB], f32, tag="l1c")
            r1 = sb.tile([P, NB], f32, tag="r1")
            l2h = sb.tile([P, NB, 2], bf16, tag="l2h")     # [tp, tb, s-half]
            l2 = sb.tile([P, NB], f32, tag="l2")
            r2 = sb.tile([P, NB], f32, tag="r2")

            make_identity(nc, ident[:])

            # ---- PSUM tiles (8 banks total) ----
            sA = [ps.tile([P, PW], f32, tag="A0", name="sA0"),
                  ps.tile([P, PW], f32, tag="A1", name="sA1")]
            o2_ps = ps.tile([P, 2, 512], f32, tag="B", name="o2_ps")
            c_ps = [ps.tile([P, 512], f32, tag="C0", name="c_ps0"),
                    ps.tile([P, 512], f32, tag="C1", name="c_ps1")]

            # ---- loads: 4-block chunks, panel-0 critical data first ----
            def load(dv, sbuf, c):
                seq("sp", nc.sync.dma_start(sbuf[:, 4 * c:4 * c + 4],
                                            dv[:, 4 * c:4 * c + 4]))

            load(x_dv, x_sb, 0)
            load(y_dv, y_sb, 0)
            load(y_dv, y_sb, 1)
            load(x_dv, x_sb, 1)
            load(x_dv, x_sb, 2)
            load(x_dv, x_sb, 3)
            load(y_dv, y_sb, 2)
            load(y_dv, y_sb, 3)

            # ---- prologue: xT/yT via PE transposes (f32, exact) ----
            # each [128,128] block transposes into a PSUM slice; drains cast
            # f32 -> f32r on ACT (idle before exps) and DVE.
            def ptr(v_sb, b, pslice):
                seq("pe", nc.tensor.transpose(pslice, v_sb[:, b, :], ident[:]))

            for b in range(4):                   # x0-3 -> A0[0:512]
                ptr(x_sb, b, sA[0][:, b * P:(b + 1) * P])
            for b in range(4):                   # y0-3 -> B[0, 0:512]
                ptr(y_sb, b, o2_ps[:, 0, b * P:(b + 1) * P])
            for b in range(4, 8):                # y4-7 -> B[1, 0:512]
                ptr(y_sb, b, o2_ps[:, 1, (b - 4) * P:(b - 3) * P])
            for b in range(4, 8):                # x4-7 -> A0[512:1024]
                ptr(x_sb, b, sA[0][:, b * P:(b + 1) * P])
            for b in range(8, 16):               # x8-15 -> A1
                ptr(x_sb, b, sA[1][:, (b - 8) * P:(b - 7) * P])
            for b in range(8, 12):               # y8-11 -> C0
                ptr(y_sb, b, c_ps[0][:, (b - 8) * P:(b - 7) * P])
            for b in range(12, 16):              # y12-15 -> C1
                ptr(y_sb, b, c_ps[1][:, (b - 12) * P:(b - 11) * P])

            # drains (f32 psum -> f32r SBUF) + bf16 operand copies.
            # ACT is idle until the first exp, DVE until the first l2 reduce;
            # GPSIMD is far too slow for bulk work (~14.5 ns/elem measured).
            def conv(eng, dst, src, c):
                sl = slice(4 * c, 4 * c + 4)
                if eng == "act":
                    seq("act", nc.scalar.copy(dst[:, sl, :], src[:, sl, :]))
                else:
                    seq("dve", nc.vector.tensor_scalar_add(
                        dst[:, sl, :], src[:, sl, :], 0.0))

            seq("act", nc.scalar.copy(xT[:, 0:4, :], sA[0][:, 0:512]))
            conv("act", x_hi, x_sb, 0)
            seq("act", nc.scalar.copy(yT[:, 0:4, :], o2_ps[:, 0, :]))
            seq("act", nc.scalar.copy(yT[:, 4:8, :], o2_ps[:, 1, :]))
            conv("act", y_hi, y_sb, 0)
            conv("act", y_hi, y_sb, 1)
            seq("dve", nc.vector.tensor_scalar_add(xT[:, 4:8, :],
                                                   sA[0][:, 512:1024], 0.0))
            seq("dve", nc.vector.tensor_scalar_add(xT[:, 8:12, :],
                                                   sA[1][:, 0:512], 0.0))
            seq("dve", nc.vector.tensor_scalar_add(xT[:, 12:16, :],
                                                   sA[1][:, 512:1024], 0.0))
            seq("dve", nc.vector.tensor_scalar_add(yT[:, 8:12, :],
                                                   c_ps[0][:], 0.0))
            seq("dve", nc.vector.tensor_scalar_add(yT[:, 12:16, :],
                                                   c_ps[1][:], 0.0))
            conv("dve", x_hi, x_sb, 1)
            conv("dve", x_hi, x_sb, 2)
            conv("dve", x_hi, x_sb, 3)
            conv("dve", y_hi, y_sb, 2)
            conv("dve", y_hi, y_sb, 3)

            yT_f = yT[:].rearrange("p b d -> p (b d)")

            # ---- emission helpers ----
            def s_block(ct, i):
                c0 = ct * PW
                slot = sA[i % 2]
                seq("pe", nc.tensor.matmul(slot[:, 0:512], xT[:, i, :],
                                           yT_f[:, c0:c0 + 512],
                                           start=True, stop=True))
                seq("pe", nc.tensor.matmul(slot[:, 512:1024], xT[:, i, :],
                                           yT_f[:, c0 + 512:c0 + 1024],
                                           start=True, stop=True))

            def exp_block(ct, i):
                c0 = ct * PW
                idx = 2 * i + ct
                seq("act", nc.scalar.activation(
                    E[:, i, c0:c0 + PW], sA[i % 2][:], Exp,
                    accum_out=l1p[:, idx:idx + 1]))

            def xbar(ct, i):
                c0 = ct * PW
                seq("sp", nc.sync.dma_start_transpose(
                    ET[:, ct * PB:(ct + 1) * PB, i * P:(i + 1) * P],
                    E[:, i, c0:c0 + PW]))

            def o2_block(ct, i):
                c0 = ct * PW
                for q in range(2):
                    seq("pe", nc.tensor.matmul(
                        o2_ps[:, q, :], x_hi[:, i, :],
                        E[:, i, c0 + q * 512:c0 + (q + 1) * 512],
                        start=(i == 0), stop=(i == NB - 1)))

            def o1_burst(ct, j, bank):
                # s-chunk j (cols j*512..), contract tb over panel ct's blocks
                for k in range(PB):
                    tb = ct * PB + k
                    seq("pe", nc.tensor.matmul(
                        bank[:], y_hi[:, tb, :],
                        ET[:, tb, j * 512:(j + 1) * 512],
                        start=(k == 0), stop=(k == PB - 1)))

            def o1_drain(ct, j, bank):
                dst = o1T_sb[:, j * 512:(j + 1) * 512]
                if ct == 0:
                    seq("dve", nc.vector.tensor_scalar_add(dst, bank[:], 0.0))
                else:
                    seq("dve", nc.vector.tensor_tensor(dst, bank[:], dst, op=ADD))

            def l1_chunk(j):
                # combine panel sums + reciprocal for s-blocks 4j..4j+3
                src = l1p[:, 8 * j:8 * j + 8].rearrange("p (i c) -> p i c", c=2)
                seq("dve", nc.vector.tensor_reduce(l1c[:, 4 * j:4 * j + 4], src,
                                                   axis=AX, op=ADD))
                seq("dve", nc.vector.reciprocal(r1[:, 4 * j:4 * j + 4],
                                                l1c[:, 4 * j:4 * j + 4]))

            def l2_half(tb, h):
                # bf16 partial sums: keeps every reduce operand 2-byte so the
                # DVE can run its 2x/4x mode; softmax sums are dominated by a
                # few large terms so bf16 accumulation noise stays ~0.4%.
                with nc.allow_low_precision(reason="bf16 l2 partials"):
                    seq("dve", nc.vector.tensor_reduce(
                        l2h[:, tb, h:h + 1],
                        ET[:, tb, h * PW:(h + 1) * PW], axis=AX, op=ADD))

            def l2_chunk(k):
                seq("dve", nc.vector.tensor_reduce(l2[:, 4 * k:4 * k + 4],
                                                   l2h[:, 4 * k:4 * k + 4, :],
                                                   axis=AX, op=ADD))
                seq("dve", nc.vector.reciprocal(r2[:, 4 * k:4 * k + 4],
                                                l2[:, 4 * k:4 * k + 4]))

            def epi(which, k, bank):
                # 4 blocks: PE transpose + DVE gate into staging, 1 store
                src_sb = o1T_sb if which == 1 else o2T_sb
                rcp = r1 if which == 1 else r2
                gate = x_sb if which == 1 else y_sb
                col0 = 0 if which == 1 else D
                st = stg.tile([P, 4, D], f32, tag="st", name=f"st{which}_{k}")
                for q in range(4):
                    blk = 4 * k + q
                    seq("pe", nc.tensor.transpose(
                        bank[:, q * P:(q + 1) * P],
                        src_sb[:, blk * P:(blk + 1) * P], ident[:]))
                    seq("dve", nc.vector.scalar_tensor_tensor(
                        st[:, q, :], bank[:, q * P:(q + 1) * P],
                        rcp[:, blk:blk + 1], gate[:, blk, :],
                        op0=MUL, op1=MUL))
                seq("sp", nc.sync.dma_start(
                    out_dv[:, 4 * k:4 * k + 4, col0:col0 + D], st[:]))

            # ---- main loop: 2 panels x 16 blocks, fully fused ----
            for ct in range(NP):
                for i in range(NB):
                    s_block(ct, i)
                    if i >= 1:
                        o2_block(ct, i - 1)
                    if ct == 1:
                        if i == 0:
                            o2_block(0, NB - 1)       # finish panel-0 o2
                            # drain panel-0 o2 before o2(1,0) reuses the bank
                            seq("dve", nc.vector.tensor_scalar_add(
                                o2T_sb[:, 0:PW],
                                o2_ps[:].rearrange("p a b -> p (a b)"), 0.0))
                        if i == 1:
                            o1_burst(0, 3, c_ps[1])   # last h0 burst
                        if i == 2:
                            o1_drain(0, 3, c_ps[1])
                        if i == 3:
                            # l2 second halves for panel-0 tb's, then r2
                            for kk in range(PB):
                                l2_half(kk, 1)
                            l2_chunk(0)
                            l2_chunk(1)
                        if i == 5:
                            epi(2, 0, c_ps[1])
                        if i == 9:
                            epi(2, 1, c_ps[1])
                    if i in (6, 10, 14):
                        j = (i - 6) // 4
                        o1_burst(ct, j, c_ps[j % 2])
                    if i in (7, 11, 15):
                        j = (i - 7) // 4
                        o1_drain(ct, j, c_ps[j % 2])
                        if ct == 1:
                            l1_chunk(j)
                            epi(1, j, c_ps[j % 2])
                    if i >= 8:
                        # l2 first halves for this panel's tb's
                        l2_half(ct * PB + (i - 8), 0)
                    exp_block(ct, i)
                    xbar(ct, i)

            # ---- tail ----
            o2_block(1, NB - 1)
            seq("dve", nc.vector.tensor_scalar_add(
                o2T_sb[:, PW:S], o2_ps[:].rearrange("p a b -> p (a b)"), 0.0))
            o1_burst(1, 3, c_ps[1])
            o1_drain(1, 3, c_ps[1])
            l1_chunk(3)
            for kk in range(PB):
                l2_half(PB + kk, 1)
            l2_chunk(2)
            l2_chunk(3)
            epi(1, 3, c_ps[1])
            epi(2, 2, c_ps[0])
            epi(2, 3, c_ps[0])

    nc.compile()
    return nc


def _get_nc():
    global _NC_CACHE
    if _NC_CACHE is None:
        nc = bacc.Bacc("TRN2", target_bir_lowering=False, debug=False,
                       num_devices=B)
        _NC_CACHE = _build_program(nc)
    return _NC_CACHE


def kernel(x, y):
    global LAST_EXEC_NS
    nc = _get_nc()
    x = np.asarray(x, dtype=np.float32)
    y = np.asarray(y, dtype=np.float32)
    in_maps = [
        {"x": np.ascontiguousarray(x[b]), "y": np.ascontiguousarray(y[b])}
        for b in range(B)
    ]
    trace = bool(int(os.environ.get("KERNEL_TRACE", "0")))
    res = run_bass_kernel_spmd(nc, in_maps, list(range(B)), trace=trace)
    LAST_EXEC_NS = res.exec_time_ns
    return np.stack([res.results[b]["out"] for b in range(B)], axis=0)


# revision 12
# speedup vs baseline: 1.2150x; 1.0982x over previous
"""BiModal attention kernel for Trainium2 (8 NeuronCores, data-parallel over batch).

Per core (one batch b): x, y: [2048, 128] fp32.
  S = x @ y.T                    (float32r matmuls, [2048, 2048])
  E = exp(S)                     (unshifted; softmax is shift-invariant and
                                  |S| <~ 67 so exp stays in fp32/bf16 range)
  a1 = (E @ y) / rowsum(E) * x
  a2 = (E.T @ x) / colsum(E) * y
  out = concat([a1, a2], -1)     ([2048, 256])

Layout: rows are relabeled r = 16*p + b (p = SBUF partition, b = block index)
so every DRAM transfer is contiguous per partition; the relabeling is applied
consistently to s and t everywhere, so the math is unchanged.

v5 schedule: one fused pipeline; every matmul is 1024 wide so each stationary
(LDWEIGHTS ~150ns, serial on this HW) covers 1024 output columns.
 - Loads split across the two HWDGE queues (x on sync, y on scalar).
 - x^T/y^T via PE transposes (f32, exact) during the load stream (HAM warmup).
 - Main loop (2 column panels x 16 row blocks): S (f32r, 1024-wide) -> exp on
   ACT (1024-wide, fused row-sum accum -> l1) -> DMA-xbar transpose of E into
   ET.  o2 accumulates 1024-wide one block behind exp; o1 runs as four
   8-deep x 1024-wide phases (panel-half contractions) placed where their ET
   columns are ready; epilogues (PE re-transpose + DVE gating + 4-wide
   stores) stream inside panel 1.
 - l2 column sums on DVE (this DVE runs 1 elem/cycle; no fast modes), split
   per tb into [0:1024], [1024:1536], [1536:2048] partials so only the last
   quarter lands after its panel ends.
PSUM (8 banks): S rotation 2x[128,1024] (4) + o2 accum [128,1024] (2) +
one tag-C region [128,1024] (2) shared serially by prologue transposes, o1
phases and epilogue transposes.
"""
import sys

sys.path.insert(0, "/opt/trn_rl_repo")

import os
import numpy as np

import concourse.bass as bass
import concourse.mybir as mybir
import concourse.tile as tile
from concourse.tile_rust import add_dep_helper
from concourse import bacc
from concourse.bass_utils import run_bass_kernel_spmd
from concourse.masks import make_identity

f32 = mybir.dt.float32
f32r = mybir.dt.float32r
bf16 = mybir.dt.bfloat16

B = 8
S = 2048
D = 128
P = 128
NB = S // P          # 16 row/col blocks
NP = 2               # column panels
PW = S // NP         # panel width (1024)
PB = PW // P         # blocks per panel (8)

_NC_CACHE = None
LAST_EXEC_NS = None


def _build_program(nc):
    x_d = nc.dram_tensor("x", [S, D], f32, kind="ExternalInput").ap()
    y_d = nc.dram_tensor("y", [S, D], f32, kind="ExternalInput").ap()
    out_d = nc.dram_tensor("out", [S, 2 * D], f32, kind="ExternalOutput").ap()

    x_dv = x_d.rearrange("(p b) d -> p b d", p=P)      # [128, 16, 128]
    y_dv = y_d.rearrange("(p b) d -> p b d", p=P)
    out_dv = out_d.rearrange("(p b) c -> p b c", p=P)  # [128, 16, 256]

    Exp = mybir.ActivationFunctionType.Exp
    MUL = mybir.AluOpType.mult
    ADD = mybir.AluOpType.add
    AX = mybir.AxisListType.X

    # per-engine no-sync order chains: engine streams execute exactly in
    # emission order, so the interleaving below is the schedule.
    last = {}

    def seq(key, inst):
        prev = last.get(key)
        if prev is not None:
            add_dep_helper(inst.ins, prev.ins, sync=False, reason="order")
        last[key] = inst
        return inst

    with tile.TileContext(nc) as tc:
        with (
            tc.tile_pool(name="sb", bufs=1) as sb,
            tc.tile_pool(name="stg", bufs=4) as stg,
            tc.tile_pool(name="ps", bufs=1, space="PSUM") as ps,
        ):
            # ---- persistent SBUF tensors ----
            x_sb = sb.tile([P, NB, D], f32, tag="x_sb")
            y_sb = sb.tile([P, NB, D], f32, tag="y_sb")
            xT = sb.tile([P, NB, P], f32r, tag="xT")       # [d, sb, sp]
            yT = sb.tile([P, NB, P], f32r, tag="yT")       # [d, tb, tp]
            x_hi = sb.tile([P, NB, D], bf16, tag="x_hi")
            y_hi = sb.tile([P, NB, D], bf16, tag="y_hi")
            E = sb.tile([P, NB, S], bf16, tag="E")         # [sp, sb, t-pos]
            ET = sb.tile([P, NB, S], bf16, tag="ET")       # [tp, tb, s-pos]
            o1T_sb = sb.tile([P, S], f32, tag="o1T")       # [d, s-pos]
            o2T_sb = sb.tile([P, S], f32, tag="o2T")       # [d, t-pos]
            ident = sb.tile([P, P], f32, tag="ident")
            l1p = sb.tile([P, 2 * NB], f32, tag="l1p")     # [sp, 2*i+ct]
            l1c = sb.tile([P, NB], f32, tag="l1c")
            r1 = sb.tile([P, NB], f32, tag="r1")
            l2h = sb.tile([P, NB, 3], f32, tag="l2h")      # [tp, tb, part]
            l2 = sb.tile([P, NB], f32, tag="l2")
            r2 = sb.tile([P, NB], f32, tag="r2")

            make_identity(nc, ident[:])

            # ---- PSUM tiles (8 banks) ----
            sA = [ps.tile([P, PW], f32, tag="A0", name="sA0"),
                  ps.tile([P, PW], f32, tag="A1", name="sA1")]
            o2_ps = ps.tile([P, PW], f32, tag="B", name="o2_ps")
            cpro = ps.tile([P, PW], f32, tag="C", name="cpro")

            def c_tile(name, shape=None):
                return ps.tile(shape or [P, PW], f32, tag="C", name=name)

            # ---- loads: x on the sync queue, y on the scalar queue ----
            for c in range(4):
                seq("sp", nc.sync.dma_start(x_sb[:, 4 * c:4 * c + 4],
                                            x_dv[:, 4 * c:4 * c + 4]))
            for c in range(4):
                seq("actq", nc.scalar.dma_start(y_sb[:, 4 * c:4 * c + 4],
                                                y_dv[:, 4 * c:4 * c + 4]))

            # ---- prologue: xT/yT via PE transposes (f32, exact) ----
            def ptr(v_sb, b, pslice):
                seq("pe", nc.tensor.transpose(pslice, v_sb[:, b, :], ident[:]))

            for b in range(4):                   # x0-3 -> A0[0:512]
                ptr(x_sb, b, sA[0][:, b * P:(b + 1) * P])
            for b in range(4):                   # y0-3 -> B[0:512]
                ptr(y_sb, b, o2_ps[:, b * P:(b + 1) * P])
            for b in range(4, 8):                # y4-7 -> B[512:1024]
                ptr(y_sb, b, o2_ps[:, b * P:(b + 1) * P])
            for b in range(4, 8):                # x4-7 -> A0[512:1024]
                ptr(x_sb, b, sA[0][:, b * P:(b + 1) * P])
            for b in range(8, 16):               # x8-15 -> A1
                ptr(x_sb, b, sA[1][:, (b - 8) * P:(b - 7) * P])
            for b in range(8, 16):               # y8-15 -> C
                ptr(y_sb, b, cpro[:, (b - 8) * P:(b - 7) * P])

            # drains (f32 psum -> f32r SBUF) + bf16 operand copies.
            def conv(eng, dst, src, c):
                sl = slice(4 * c, 4 * c + 4)
                if eng == "act":
                    seq("act", nc.scalar.copy(dst[:, sl, :], src[:, sl, :]))
                else:
                    seq("dve", nc.vector.tensor_scalar_add(
                        dst[:, sl, :], src[:, sl, :], 0.0))

            seq("act", nc.scalar.copy(xT[:, 0:4, :], sA[0][:, 0:512]))
            conv("act", x_hi, x_sb, 0)
            seq("act", nc.scalar.copy(yT[:, 0:4, :], o2_ps[:, 0:512]))
            seq("act", nc.scalar.copy(yT[:, 4:8, :], o2_ps[:, 512:1024]))
            conv("act", y_hi, y_sb, 0)
            conv("act", y_hi, y_sb, 1)
            seq("dve", nc.vector.tensor_scalar_add(xT[:, 4:8, :],
                                                   sA[0][:, 512:1024], 0.0))
            seq("dve", nc.vector.tensor_scalar_add(xT[:, 8:12, :],
                                                   sA[1][:, 0:512], 0.0))
            seq("dve", nc.vector.tensor_scalar_add(xT[:, 12:16, :],
                                                   sA[1][:, 512:1024], 0.0))
            seq("dve", nc.vector.tensor_scalar_add(yT[:, 8:12, :],
                                                   cpro[:, 0:512], 0.0))
            seq("dve", nc.vector.tensor_scalar_add(yT[:, 12:16, :],
                                                   cpro[:, 512:1024], 0.0))
            conv("dve", x_hi, x_sb, 1)
            conv("dve", x_hi, x_sb, 2)
            conv("dve", x_hi, x_sb, 3)
            conv("dve", y_hi, y_sb, 2)
            conv("dve", y_hi, y_sb, 3)

            yT_f = yT[:].rearrange("p b d -> p (b d)")

            # ---- emission helpers ----
            def s_block(ct, i):
                c0 = ct * PW
                for q in range(2):
                    seq("pe", nc.tensor.matmul(
                        sA[i % 2][:, q * 512:(q + 1) * 512], xT[:, i, :],
                        yT_f[:, c0 + q * 512:c0 + (q + 1) * 512],
                        start=True, stop=True))

            def exp_block(ct, i):
                c0 = ct * PW
                idx = 2 * i + ct
                seq("act", nc.scalar.activation(
                    E[:, i, c0:c0 + PW], sA[i % 2][:], Exp,
                    accum_out=l1p[:, idx:idx + 1]))

            def xbar(ct, i):
                c0 = ct * PW
                seq("sp", nc.sync.dma_start_transpose(
                    ET[:, ct * PB:(ct + 1) * PB, i * P:(i + 1) * P],
                    E[:, i, c0:c0 + PW]))

            def o2_block(ct, i):
                c0 = ct * PW
                for q in range(2):
                    seq("pe", nc.tensor.matmul(
                        o2_ps[:, q * 512:(q + 1) * 512], x_hi[:, i, :],
                        E[:, i, c0 + q * 512:c0 + (q + 1) * 512],
                        start=(i == 0), stop=(i == NB - 1)))

            def o1_phase_mm(ct, k2, cb):
                # tb-contraction steps of o1 half cb["half"] on psum cb
                h0 = cb["half"] * PW
                for k in k2:
                    tb = ct * PB + k
                    for q in range(2):
                        seq("pe", nc.tensor.matmul(
                            cb["ap"][:, q * 512:(q + 1) * 512], y_hi[:, tb, :],
                            ET[:, tb, h0 + q * 512:h0 + (q + 1) * 512],
                            start=(k == 0), stop=(k == PB - 1)))

            def o1_phase_drain(ct, cb):
                dst = o1T_sb[:, cb["half"] * PW:(cb["half"] + 1) * PW]
                if ct == 0:
                    seq("dve", nc.vector.tensor_scalar_add(dst, cb["ap"][:], 0.0))
                else:
                    seq("dve", nc.vector.tensor_tensor(dst, cb["ap"][:], dst,
                                                       op=ADD))

            def l1_chunk(j):
                src = l1p[:, 8 * j:8 * j + 8].rearrange("p (i c) -> p i c", c=2)
                seq("dve", nc.vector.tensor_reduce(l1c[:, 4 * j:4 * j + 4], src,
                                                   axis=AX, op=ADD))
                seq("dve", nc.vector.reciprocal(r1[:, 4 * j:4 * j + 4],
                                                l1c[:, 4 * j:4 * j + 4]))

            def l2_part(tb, part):
                # part 0: s-cols [0:1024); 1: [1024:1536); 2: [1536:2048)
                lo = (0, PW, PW + 512)[part]
                hi = (PW, PW + 512, S)[part]
                seq("dve", nc.vector.tensor_reduce(
                    l2h[:, tb, part:part + 1], ET[:, tb, lo:hi],
                    axis=AX, op=ADD))

            def l2_chunk(k):
                seq("dve", nc.vector.tensor_reduce(l2[:, 4 * k:4 * k + 4],
                                                   l2h[:, 4 * k:4 * k + 4, :],
                                                   axis=AX, op=ADD))
                seq("dve", nc.vector.reciprocal(r2[:, 4 * k:4 * k + 4],
                                                l2[:, 4 * k:4 * k + 4]))

            def epi(which, k):
                # 4 blocks: PE transpose + DVE gate into staging, 1 store
                src_sb = o1T_sb if which == 1 else o2T_sb
                rcp = r1 if which == 1 else r2
                gate = x_sb if which == 1 else y_sb
                col0 = 0 if which == 1 else D
                e_ps = c_tile(f"e{which}_{k}", [P, 512])
                st = stg.tile([P, 4, D], f32, tag="st", name=f"st{which}_{k}")
                for q in range(4):
                    blk = 4 * k + q
                    seq("pe", nc.tensor.transpose(
                        e_ps[:, q * P:(q + 1) * P],
                        src_sb[:, blk * P:(blk + 1) * P], ident[:]))
                    seq("dve", nc.vector.scalar_tensor_tensor(
                        st[:, q, :], e_ps[:, q * P:(q + 1) * P],
                        rcp[:, blk:blk + 1], gate[:, blk, :],
                        op0=MUL, op1=MUL))
                seq("sp", nc.sync.dma_start(
                    out_dv[:, 4 * k:4 * k + 4, col0:col0 + D], st[:]))

            # ---- main loop ----
            # o1 phases: (panel ct, s-half h) contracts panel ct's 8 tb blocks
            # over ET columns h*1024..; placed where those columns are ready.
            phA0 = {"half": 0, "ap": None}   # p0 tbs, cols 0:1024
            phB0 = {"half": 1, "ap": None}   # p0 tbs, cols 1024:2048
            phA1 = {"half": 0, "ap": None}   # p1 tbs, cols 0:1024
            phB1 = {"half": 1, "ap": None}   # p1 tbs, cols 1024:2048

            for ct in range(NP):
                for i in range(NB):
                    s_block(ct, i)
                    if i >= 1:
                        o2_block(ct, i - 1)
                    if ct == 0:
                        if i == 10:
                            phA0["ap"] = c_tile("c_phA0")
                        if 10 <= i <= 13:
                            o1_phase_mm(0, (2 * (i - 10), 2 * (i - 10) + 1),
                                        phA0)
                        if i == 14:
                            o1_phase_drain(0, phA0)
                        if i >= 8:
                            l2_part(i - 8, 0)
                        if i == 14:
                            for tb in range(0, 4):
                                l2_part(tb, 1)
                        if i == 15:
                            for tb in range(4, 8):
                                l2_part(tb, 1)
                    else:
                        if i == 0:
                            o2_block(0, NB - 1)
                            seq("dve", nc.vector.tensor_scalar_add(
                                o2T_sb[:, 0:PW], o2_ps[:], 0.0))
                        if i == 1:
                            phB0["ap"] = c_tile("c_phB0")
                        if 1 <= i <= 4:
                            o1_phase_mm(0, (2 * (i - 1), 2 * (i - 1) + 1),
                                        phB0)
                            for tb in (2 * (i - 1), 2 * (i - 1) + 1):
                                l2_part(tb, 2)
                        if i == 5:
                            o1_phase_drain(0, phB0)
                            l2_chunk(0)
                            l2_chunk(1)
                        if i == 6:
                            epi(2, 0)
                        if i == 8:
                            epi(2, 1)
                        if i == 11:
                            phA1["ap"] = c_tile("c_phA1")
                        if 11 <= i <= 14:
                            o1_phase_mm(1, (2 * (i - 11), 2 * (i - 11) + 1),
                                        phA1)
                        if i >= 8:
                            l2_part(PB + (i - 8), 0)
                        if i == 14:
                            for tb in range(8, 12):
                                l2_part(tb, 1)
                        if i == 15:
                            for tb in range(12, 16):
                                l2_part(tb, 1)
                            o1_phase_drain(1, phA1)
                            l1_chunk(0)
                            l1_chunk(1)
                            epi(1, 0)
                    exp_block(ct, i)
                    xbar(ct, i)

            # ---- tail ----
            o2_block(1, NB - 1)
            seq("dve", nc.vector.tensor_scalar_add(
                o2T_sb[:, PW:S], o2_ps[:], 0.0))
            epi(1, 1)
            phB1["ap"] = c_tile("c_phB1")
            o1_phase_mm(1, (0, 1, 2, 3), phB1)
            for tb in range(8, 12):
                l2_part(tb, 2)
            o1_phase_mm(1, (4, 5, 6, 7), phB1)
            for tb in range(12, 16):
                l2_part(tb, 2)
            o1_phase_drain(1, phB1)
            l1_chunk(2)
            l1_chunk(3)
            l2_chunk(2)
            l2_chunk(3)
            epi(1, 2)
            epi(2, 2)
            epi(1, 3)
            epi(2, 3)

    nc.compile()
    return nc


def _get_nc():
    global _NC_CACHE
    if _NC_CACHE is None:
        nc = bacc.Bacc("TRN2", target_bir_lowering=False, debug=False,
                       num_devices=B)
        _NC_CACHE = _build_program(nc)
    return _NC_CACHE


def kernel(x, y):
    global LAST_EXEC_NS
    nc = _get_nc()
    x = np.asarray(x, dtype=np.float32)
    y = np.asarray(y, dtype=np.float32)
    in_maps = [
        {"x": np.ascontiguousarray(x[b]), "y": np.ascontiguousarray(y[b])}
        for b in range(B)
    ]
    trace = bool(int(os.environ.get("KERNEL_TRACE", "0")))
    res = run_bass_kernel_spmd(nc, in_maps, list(range(B)), trace=trace)
    LAST_EXEC_NS = res.exec_time_ns
    return np.stack([res.results[b]["out"] for b in range(B)], axis=0)


# revision 17
# speedup vs baseline: 1.3949x; 1.1480x over previous
"""BiModal attention kernel for Trainium2 (8 NeuronCores, data-parallel over batch).

Per core (one batch b): x, y: [2048, 128] fp32.
  S = x @ y.T                    (float32r matmuls, [2048, 2048])
  E = exp(S)                     (unshifted; softmax is shift-invariant and
                                  |S| <~ 67 so exp stays in fp32/bf16 range)
  a1 = (E @ y) / rowsum(E) * x
  a2 = (E.T @ x) / colsum(E) * y
  out = concat([a1, a2], -1)     ([2048, 256])

Layout: rows are relabeled r = 16*p + b (p = SBUF partition, b = block index)
so every DRAM transfer is contiguous per partition; the relabeling is applied
consistently to s and t everywhere, so the math is unchanged.

v7: PE row-work is the wall (~0.75 ns/row effective; 512-wide matmul ~390ns),
so the schedule keeps the PE queue non-empty and moves everything else off it:
 - x^T/y^T via PE transposes (f32, exact) during the 2-queue load stream.
 - Main loop (2 panels x 16 blocks): S (f32r 2x512) -> exp (ACT, 1024-wide,
   accum -> l1) -> DMA-xbar E -> ET.  o2 (bf16 2x512) one block behind; o1 as
   eight 8-deep x 512-wide quarter-bursts placed right after their ET columns
   land, so only the last quarter trails the loop.
 - Epilogue transposes go through the DMA xbar (o1T/o2T kept in bf16), not
   the PE: transposed halves -> o1N/o2N, then DVE gating + 4-wide stores.
 - l2 column sums on DVE (1 elem/cycle; no fast modes on this DVE), split
   [0:1024]/[1024:1536]/[1536:2048] per tb; DVE emission order puts
   PE-gating drains ahead of bulk l2 work.
PSUM (8 banks): S rotation 2x[128,1024] (4) + o2 accum [128,1024] (2) + o1
quarter-burst region [128,1024] (2, halves alternate).
"""
import sys

sys.path.insert(0, "/opt/trn_rl_repo")

import os
import numpy as np

import concourse.bass as bass
import concourse.mybir as mybir
import concourse.tile as tile
from concourse.tile_rust import add_dep_helper
from concourse import bacc
from concourse.bass_utils import run_bass_kernel_spmd
from concourse.masks import make_identity

f32 = mybir.dt.float32
f32r = mybir.dt.float32r
bf16 = mybir.dt.bfloat16

B = 8
S = 2048
D = 128
P = 128
NB = S // P          # 16 row/col blocks
NP = 2               # column panels
PW = S // NP         # panel width (1024)
PB = PW // P         # blocks per panel (8)

_NC_CACHE = None
LAST_EXEC_NS = None


def _build_program(nc):
    x_d = nc.dram_tensor("x", [S, D], f32, kind="ExternalInput").ap()
    y_d = nc.dram_tensor("y", [S, D], f32, kind="ExternalInput").ap()
    out_d = nc.dram_tensor("out", [S, 2 * D], f32, kind="ExternalOutput").ap()

    x_dv = x_d.rearrange("(p b) d -> p b d", p=P)      # [128, 16, 128]
    y_dv = y_d.rearrange("(p b) d -> p b d", p=P)
    out_dv = out_d.rearrange("(p b) c -> p b c", p=P)  # [128, 16, 256]

    Exp = mybir.ActivationFunctionType.Exp
    MUL = mybir.AluOpType.mult
    ADD = mybir.AluOpType.add
    AX = mybir.AxisListType.X

    last = {}

    def seq(key, inst):
        prev = last.get(key)
        if prev is not None:
            add_dep_helper(inst.ins, prev.ins, sync=False, reason="order")
        last[key] = inst
        return inst

    with tile.TileContext(nc) as tc:
        with (
            tc.tile_pool(name="sb", bufs=1) as sb,
            tc.tile_pool(name="stg", bufs=4) as stg,
            tc.tile_pool(name="ps", bufs=1, space="PSUM") as ps,
        ):
            # ---- persistent SBUF tensors ----
            x_sb = sb.tile([P, NB, D], f32, tag="x_sb")
            y_sb = sb.tile([P, NB, D], f32, tag="y_sb")
            xT = sb.tile([P, NB, P], f32r, tag="xT")       # [d, sb, sp]
            yT = sb.tile([P, NB, P], f32r, tag="yT")       # [d, tb, tp]
            x_hi = sb.tile([P, NB, D], bf16, tag="x_hi")
            y_hi = sb.tile([P, NB, D], bf16, tag="y_hi")
            E = sb.tile([P, NB, S], bf16, tag="E")         # [sp, sb, t-pos]
            ET = sb.tile([P, NB, S], bf16, tag="ET")       # [tp, tb, s-pos]
            o1T_sb = sb.tile([P, S], bf16, tag="o1T")      # [d, s-pos]
            o2T_sb = sb.tile([P, S], bf16, tag="o2T")      # [d, t-pos]
            o1N = sb.tile([P, NB, D], bf16, tag="o1N")     # [sp, sb, d]
            o2N = sb.tile([P, NB, D], bf16, tag="o2N")     # [tp, tb, d]
            ident = sb.tile([P, P], f32, tag="ident")
            l1p = sb.tile([P, 2 * NB], f32, tag="l1p")     # [sp, 2*i+ct]
            l1c = sb.tile([P, NB], f32, tag="l1c")
            r1 = sb.tile([P, NB], f32, tag="r1")
            l2h = sb.tile([P, NB, 3], f32, tag="l2h")      # [tp, tb, part]
            l2 = sb.tile([P, NB], f32, tag="l2")
            r2 = sb.tile([P, NB], f32, tag="r2")

            make_identity(nc, ident[:])

            # ---- PSUM tiles (8 banks) ----
            sA = [ps.tile([P, PW], f32, tag="A0", name="sA0"),
                  ps.tile([P, PW], f32, tag="A1", name="sA1")]
            o2_ps = ps.tile([P, PW], f32, tag="B", name="o2_ps")
            c_ps = ps.tile([P, PW], f32, tag="C", name="c_ps")

            # ---- loads: x on the sync queue, y on the scalar queue ----
            for c in range(4):
                seq("sp", nc.sync.dma_start(x_sb[:, 4 * c:4 * c + 4],
                                            x_dv[:, 4 * c:4 * c + 4]))
            for c in range(4):
                seq("actq", nc.scalar.dma_start(y_sb[:, 4 * c:4 * c + 4],
                                                y_dv[:, 4 * c:4 * c + 4]))

            # ---- prologue: xT/yT via PE transposes (f32, exact) ----
            def ptr(v_sb, b, pslice):
                seq("pe", nc.tensor.transpose(pslice, v_sb[:, b, :], ident[:]))

            for b in range(4):                   # x0-3 -> A0[0:512]
                ptr(x_sb, b, sA[0][:, b * P:(b + 1) * P])
            for b in range(4):                   # y0-3 -> B[0:512]
                ptr(y_sb, b, o2_ps[:, b * P:(b + 1) * P])
            for b in range(4, 8):                # y4-7 -> B[512:1024]
                ptr(y_sb, b, o2_ps[:, b * P:(b + 1) * P])
            for b in range(4, 8):                # x4-7 -> A0[512:1024]
                ptr(x_sb, b, sA[0][:, b * P:(b + 1) * P])
            for b in range(8, 16):               # x8-15 -> A1
                ptr(x_sb, b, sA[1][:, (b - 8) * P:(b - 7) * P])
            for b in range(8, 16):               # y8-15 -> C
                ptr(y_sb, b, c_ps[:, (b - 8) * P:(b - 7) * P])

            # drains (f32 psum -> f32r SBUF) + bf16 operand copies
            def conv(eng, dst, src, c):
                sl = slice(4 * c, 4 * c + 4)
                if eng == "act":
                    seq("act", nc.scalar.copy(dst[:, sl, :], src[:, sl, :]))
                else:
                    seq("dve", nc.vector.tensor_scalar_add(
                        dst[:, sl, :], src[:, sl, :], 0.0))

            seq("act", nc.scalar.copy(xT[:, 0:4, :], sA[0][:, 0:512]))
            conv("act", x_hi, x_sb, 0)
            seq("act", nc.scalar.copy(yT[:, 0:4, :], o2_ps[:, 0:512]))
            seq("act", nc.scalar.copy(yT[:, 4:8, :], o2_ps[:, 512:1024]))
            conv("act", y_hi, y_sb, 0)
            conv("act", y_hi, y_sb, 1)
            seq("dve", nc.vector.tensor_scalar_add(xT[:, 4:8, :],
                                                   sA[0][:, 512:1024], 0.0))
            seq("dve", nc.vector.tensor_scalar_add(xT[:, 8:12, :],
                                                   sA[1][:, 0:512], 0.0))
            seq("dve", nc.vector.tensor_scalar_add(xT[:, 12:16, :],
                                                   sA[1][:, 512:1024], 0.0))
            seq("dve", nc.vector.tensor_scalar_add(yT[:, 8:12, :],
                                                   c_ps[:, 0:512], 0.0))
            seq("dve", nc.vector.tensor_scalar_add(yT[:, 12:16, :],
                                                   c_ps[:, 512:1024], 0.0))
            conv("dve", x_hi, x_sb, 1)
            conv("dve", x_hi, x_sb, 2)
            conv("dve", x_hi, x_sb, 3)
            conv("dve", y_hi, y_sb, 2)
            conv("dve", y_hi, y_sb, 3)

            yT_f = yT[:].rearrange("p b d -> p (b d)")

            # ---- emission helpers ----
            def s_block(ct, i):
                c0 = ct * PW
                for q in range(2):
                    seq("pe", nc.tensor.matmul(
                        sA[i % 2][:, q * 512:(q + 1) * 512], xT[:, i, :],
                        yT_f[:, c0 + q * 512:c0 + (q + 1) * 512],
                        start=True, stop=True))

            def exp_block(ct, i):
                c0 = ct * PW
                idx = 2 * i + ct
                seq("act", nc.scalar.activation(
                    E[:, i, c0:c0 + PW], sA[i % 2][:], Exp,
                    accum_out=l1p[:, idx:idx + 1]))

            def xbar(ct, i):
                c0 = ct * PW
                seq("sp", nc.sync.dma_start_transpose(
                    ET[:, ct * PB:(ct + 1) * PB, i * P:(i + 1) * P],
                    E[:, i, c0:c0 + PW]))

            def o2_block(ct, i):
                c0 = ct * PW
                for q in range(2):
                    seq("pe", nc.tensor.matmul(
                        o2_ps[:, q * 512:(q + 1) * 512], x_hi[:, i, :],
                        E[:, i, c0 + q * 512:c0 + (q + 1) * 512],
                        start=(i == 0), stop=(i == NB - 1)))

            def o1_q_mm(ct, j, ks):
                # o1 quarter j (s-cols j*512..): contract tb over panel ct
                cb = c_ps[:, (j % 2) * 512:(j % 2) * 512 + 512]
                for k in ks:
                    tb = ct * PB + k
                    seq("pe", nc.tensor.matmul(
                        cb, y_hi[:, tb, :],
                        ET[:, tb, j * 512:(j + 1) * 512],
                        start=(k == 0), stop=(k == PB - 1)))

            def o1_q_drain(ct, j):
                cb = c_ps[:, (j % 2) * 512:(j % 2) * 512 + 512]
                dst = o1T_sb[:, j * 512:(j + 1) * 512]
                with nc.allow_low_precision(reason="bf16 o1 staging"):
                    if ct == 0:
                        seq("dve", nc.vector.tensor_scalar_add(dst, cb, 0.0))
                    else:
                        seq("dve", nc.vector.tensor_tensor(dst, cb, dst,
                                                           op=ADD))

            def l1_chunk(j):
                src = l1p[:, 8 * j:8 * j + 8].rearrange("p (i c) -> p i c", c=2)
                seq("dve", nc.vector.tensor_reduce(l1c[:, 4 * j:4 * j + 4], src,
                                                   axis=AX, op=ADD))
                seq("dve", nc.vector.reciprocal(r1[:, 4 * j:4 * j + 4],
                                                l1c[:, 4 * j:4 * j + 4]))

            def l2_part(tb, part):
                lo = (0, PW, PW + 512)[part]
                hi = (PW, PW + 512, S)[part]
                seq("dve", nc.vector.tensor_reduce(
                    l2h[:, tb, part:part + 1], ET[:, tb, lo:hi],
                    axis=AX, op=ADD))

            def l2_chunk(k):
                seq("dve", nc.vector.tensor_reduce(l2[:, 4 * k:4 * k + 4],
                                                   l2h[:, 4 * k:4 * k + 4, :],
                                                   axis=AX, op=ADD))
                seq("dve", nc.vector.reciprocal(r2[:, 4 * k:4 * k + 4],
                                                l2[:, 4 * k:4 * k + 4]))

            def epi_xbar(which, half):
                # transpose a finished [128, 1024] half of o1T/o2T into
                # natural layout via the DMA xbar
                src = o1T_sb if which == 1 else o2T_sb
                dst = o1N if which == 1 else o2N
                seq("sp", nc.sync.dma_start_transpose(
                    dst[:, half * PB:(half + 1) * PB, :],
                    src[:, half * PW:(half + 1) * PW]))

            def epi(which, k):
                # 4 blocks: DVE gate into staging, 1 store
                srcN = o1N if which == 1 else o2N
                rcp = r1 if which == 1 else r2
                gate = x_sb if which == 1 else y_sb
                col0 = 0 if which == 1 else D
                st = stg.tile([P, 4, D], f32, tag="st", name=f"st{which}_{k}")
                for q in range(4):
                    blk = 4 * k + q
                    seq("dve", nc.vector.scalar_tensor_tensor(
                        st[:, q, :], srcN[:, blk, :],
                        rcp[:, blk:blk + 1], gate[:, blk, :],
                        op0=MUL, op1=MUL))
                seq("sp", nc.sync.dma_start(
                    out_dv[:, 4 * k:4 * k + 4, col0:col0 + D], st[:]))

            # ---- main loop ----
            # o1 quarters: (panel ct, quarter j) contracts panel ct's 8 tbs
            # over ET s-cols j*512..; ready ~2 slots after xbar(ct', 4j+3).
            for ct in range(NP):
                for i in range(NB):
                    s_block(ct, i)
                    if i >= 1:
                        o2_block(ct, i - 1)
                    if i in (6, 7, 10, 11, 14, 15):
                        j = (i - 6) // 4
                        o1_q_mm(ct, j, (0, 1, 2, 3) if i % 2 == 0 else
                                (4, 5, 6, 7))
                        if i % 2 == 1:
                            o1_q_drain(ct, j)
                    if ct == 0:
                        if i >= 8:
                            l2_part(i - 8, 0)
                    else:
                        if i == 0:
                            o2_block(0, NB - 1)
                            with nc.allow_low_precision(
                                    reason="bf16 o2 staging"):
                                seq("dve", nc.vector.tensor_scalar_add(
                                    o2T_sb[:, 0:PW], o2_ps[:], 0.0))
                        if i in (1, 2):
                            # last p0 o1 quarter (j=3)
                            o1_q_mm(0, 3, (0, 1, 2, 3) if i == 1 else
                                    (4, 5, 6, 7))
                            if i == 2:
                                o1_q_drain(0, 3)
                                epi_xbar(2, 0)
                        if i in (1, 2, 3, 4):
                            for tb in (2 * (i - 1), 2 * i - 1):
                                l2_part(tb, 2)
                        if i in (3, 4):
                            # p0 tb's middle partials (needed by chunk 0/1)
                            for tb in range(4 * (i - 3), 4 * (i - 3) + 4):
                                l2_part(tb, 1)
                        if i == 5:
                            l2_chunk(0)
                            l2_chunk(1)
                        if i == 6:
                            epi(2, 0)
                        if i == 8:
                            epi(2, 1)
                        if i == 9:
                            # o1 cols 0:1024 final after quarters 0,1 drained
                            l1_chunk(0)
                            l1_chunk(1)
                        if i == 12:
                            epi_xbar(1, 0)
                        if i == 13:
                            epi(1, 0)
                        if i == 15:
                            epi(1, 1)
                        if i >= 8:
                            l2_part(PB + (i - 8), 0)
                        if i == 14:
                            for tb in range(8, 12):
                                l2_part(tb, 1)
                        if i == 15:
                            for tb in range(12, 16):
                                l2_part(tb, 1)
                    exp_block(ct, i)
                    xbar(ct, i)

            # ---- tail ----
            o2_block(1, NB - 1)
            with nc.allow_low_precision(reason="bf16 o2 staging"):
                seq("dve", nc.vector.tensor_scalar_add(
                    o2T_sb[:, PW:S], o2_ps[:], 0.0))
            epi_xbar(2, 1)
            o1_q_mm(1, 3, (0, 1, 2, 3))
            for tb in range(8, 12):
                l2_part(tb, 2)
            o1_q_mm(1, 3, (4, 5, 6, 7))
            for tb in range(12, 16):
                l2_part(tb, 2)
            o1_q_drain(1, 3)
            epi_xbar(1, 1)
            l1_chunk(2)
            l1_chunk(3)
            l2_chunk(2)
            l2_chunk(3)
            epi(2, 2)
            epi(2, 3)
            epi(1, 2)
            epi(1, 3)

    nc.compile()
    return nc


def _get_nc():
    global _NC_CACHE
    if _NC_CACHE is None:
        nc = bacc.Bacc("TRN2", target_bir_lowering=False, debug=False,
                       num_devices=B)
        _NC_CACHE = _build_program(nc)
    return _NC_CACHE


def kernel(x, y):
    global LAST_EXEC_NS
    nc = _get_nc()
    x = np.asarray(x, dtype=np.float32)
    y = np.asarray(y, dtype=np.float32)
    in_maps = [
        {"x": np.ascontiguousarray(x[b]), "y": np.ascontiguousarray(y[b])}
        for b in range(B)
    ]
    trace = bool(int(os.environ.get("KERNEL_TRACE", "0")))
    res = run_bass_kernel_spmd(nc, in_maps, list(range(B)), trace=trace)
    LAST_EXEC_NS = res.exec_time_ns
    return np.stack([res.results[b]["out"] for b in range(B)], axis=0)
